# revision 1
# baseline (speedup 1.0000x reference)
"""Trainium2 Bass kernel: multi-head flash self-attention with RoPE.

Problem: x[4,2048,1024], 16 heads, dh=64, causal, RoPE(theta=10000), WO proj.

Sharding (8 cores): core c -> batch b=c//2, head-group g=c%2 (8 heads each).
Per core:
  - QKV projections of x[b] (bf16 matmuls, fp32 PSUM accumulation).
  - RoPE folded into a host-side weight-row permutation (per head: even dims
    then odd dims) so the rotation becomes tile-local partition algebra.
  - Flash attention in S^T layout ([k,q] blocks). V is stored per (ktile,
    head-pair) as [V_A | ones | V_B] so each head's stationary operand is a
    contiguous 128 cols and the softmax denominators appear as 64 replicated
    PSUM rows. No max subtraction (scores ~ N(0,1) by construction).
  - Pairwise AllGather of normalized O^T; each core then computes the output
    projection for ALL 2048 rows but only its 512 WO columns (keeps the SPMD
    program identical across cores).
Host reassembles: out[b] = concat(cols of core 2b, cols of core 2b+1).
"""
import sys

sys.path.insert(0, "/opt/trn_rl_repo")

import numpy as np
import ml_dtypes
import concourse.bass as bass
import concourse.bacc as bacc
import concourse.mybir as mybir
from concourse import tile
from concourse.bass_utils import run_bass_kernel_spmd

f32 = mybir.dt.float32
bf16 = mybir.dt.bfloat16
AF = mybir.ActivationFunctionType

S = 2048
D = 1024
H = 16
DH = 64
NCORE = 8
SL = 512           # local m dims (8 heads x 64)
NEG = -1e30
SCALE = 1.0 / 8.0  # 1/sqrt(dh)
GROUPS = [[0, 1], [2, 3], [4, 5], [6, 7]]
VPP = 192          # v_store cols per (ktile, pair): [V_A | ones | V_B]
VKT = 4 * VPP      # v_store cols per ktile


def build(timing=False):
    nc = bacc.Bacc("TRN2", target_bir_lowering=False, debug=False,
                   num_devices=1 if timing else NCORE)

    xT = nc.dram_tensor("xT", [D, S], bf16, kind="ExternalInput").ap()
    wqT = nc.dram_tensor("wqT", [D, SL], bf16, kind="ExternalInput").ap()
    wkT = nc.dram_tensor("wkT", [D, SL], bf16, kind="ExternalInput").ap()
    wvT = nc.dram_tensor("wvT", [D, SL], bf16, kind="ExternalInput").ap()
    woT = nc.dram_tensor("woT", [D, SL], bf16, kind="ExternalInput").ap()
    cosr = nc.dram_tensor("cosr", [128, S], f32, kind="ExternalInput").ap()
    sinr = nc.dram_tensor("sinr", [128, S], f32, kind="ExternalInput").ap()
    out = nc.dram_tensor("out", [S, SL], f32, kind="ExternalOutput").ap()

    og_send = [nc.dram_tensor(f"og_send{p}", [128, S], bf16) for p in range(4)]
    acc_d = [nc.dram_tensor(f"acc_d{i}", [128, SL], f32) for i in range(16)]
    og_recv = [nc.dram_tensor(f"og_recv{p}", [256, S], bf16) for p in range(4)]

    with tile.TileContext(nc) as tc:
        _body(nc, tc, xT, wqT, wkT, wvT, woT, cosr, sinr, out,
              og_send, og_recv, acc_d, timing)
    nc.compile()
    return nc


def _body(nc, tc, xT, wqT, wkT, wvT, woT, cosr, sinr, out,
          og_send, og_recv, acc_d, timing=False):
    from contextlib import ExitStack
    ctx = ExitStack()
    with ctx:
        sb = ctx.enter_context(tc.tile_pool(name="sb", bufs=1))
        psp = ctx.enter_context(tc.tile_pool(name="psp", bufs=1, space="PSUM"))
        counter = [0]

        def til(shape, dtype, tag, bufs):
            counter[0] += 1
            return sb.tile(shape, dtype, tag=tag, bufs=bufs,
                           name=f"{tag}_{counter[0]}")

        # ---------------- RoPE tables (host-computed) ----------------
        cos_t = til([128, S], f32, "cos", 1)
        nc.sync.dma_start(cos_t[:], cosr[:])
        sin_t = til([128, S], f32, "sin", 1)
        nc.sync.dma_start(sin_t[:], sinr[:])

        # 0/1 triangle mask [128,128]: 1 where c - r >= 0 (valid)
        mask_t = til([128, 128], bf16, "mask", 1)
        nc.gpsimd.memset(mask_t[:], 1.0)
        nc.gpsimd.affine_select(
            out=mask_t[:], in_=mask_t[:], compare_op=mybir.AluOpType.is_ge,
            fill=0.0, base=0, pattern=[[1, 128]], channel_multiplier=-1,
        )

        # ---------------- input loads ----------------
        def load_w(wdram, tag="w"):
            tiles = []
            for dt in range(8):
                t = til([128, SL], bf16, tag, 8)
                nc.sync.dma_start(t[:], wdram[dt * 128:(dt + 1) * 128, :])
                tiles.append(t)
            return tiles

        wv_t = load_w(wvT, "w")
        xt = []
        for dt in range(8):
            t = til([128, S], bf16, "xt", 8)
            nc.sync.dma_start(t[:], xT[dt * 128:(dt + 1) * 128, :])
            xt.append(t)
        wq_t = load_w(wqT, "wq")
        wk_t = load_w(wkT, "wk")

        # v quarters: v_q[i] holds ktiles 4i..4i+4; per (kt, pair p) block
        # of VPP cols: [V_A | ones | V_B]
        v_q = []
        for i in range(16):
            vq = til([128, VKT], bf16, "v", 16)
            nc.gpsimd.memset(vq[:], 1.0)
            v_q.append(vq)

        def emit_v_quarter(i):
            for kt4 in range(4):
                kt = 4 * i + kt4
                ps = psp.tile([128, 512], f32, tag="proj", bufs=2)
                for dt in range(8):
                    nc.tensor.matmul(
                        ps[:],
                        xt[dt][:, kt * 128:(kt + 1) * 128],
                        wv_t[dt][:],
                        start=(dt == 0), stop=(dt == 7),
                    )
                vva = v_q[kt][:].rearrange("q (a c) -> q a c", c=64)
                psa = ps[:].rearrange("q (a c) -> q a c", c=64)
                nc.vector.tensor_copy(vva[:, 0:12:3, :], psa[:, 0:8:2, :])
                nc.vector.tensor_copy(vva[:, 2:12:3, :], psa[:, 1:8:2, :])

        def v_slice(kt, p, c0, c1):
            off = p * VPP
            return v_q[kt][:, off + c0:off + c1]

        # per-st projection + rope into a [128, 512] tile
        def proj_rope_st(wtiles, mt, st, fast=False):
            big_t = til([128, 512], bf16, "qk", 18)
            ps = psp.tile([128, 512], f32, tag="proj", bufs=2)
            for dt in range(8):
                nc.tensor.matmul(
                    ps[:],
                    wtiles[dt][:, mt * 128:(mt + 1) * 128],
                    xt[dt][:, st * 512:(st + 1) * 512],
                    start=(dt == 0), stop=(dt == 7),
                )
            cols = slice(st * 512, (st + 1) * 512)
            pre_t = til([128, 512], f32, "pre", 2)
            nc.vector.tensor_copy(pre_t[:], ps[:])
            swp = til([128, 512], f32, "swp", 2)
            for a in range(4):
                srcp = (a ^ 1) * 32
                nc.sync.dma_start(swp[a * 32:(a + 1) * 32, :],
                                  pre_t[srcp:srcp + 32, :])
            tmp = til([128, 512], f32, "tmp", 2)
            nc.vector.tensor_mul(tmp[:], pre_t[:], cos_t[:, cols])
            if fast:
                nc.vector.tensor_mul(swp[:], swp[:], sin_t[:, cols])
            else:
                nc.gpsimd.tensor_mul(swp[:], swp[:], sin_t[:, cols])
            nc.vector.tensor_add(big_t[:], tmp[:], swp[:])
            return big_t

        # -------- per pair: Q/K projection + rope + flash attention --------
        for p in range(4):
            qtr = [None] * 4
            ktr = [None] * 4
            if p > 0:
                for st in range(4):
                    qtr[st] = proj_rope_st(wq_t, p, st)
                for st in range(4):
                    ktr[st] = proj_rope_st(wk_t, p, st)

            for qb in range(4):
                if p == 0:
                    emit_v_quarter(qb)
                    qtr[qb] = proj_rope_st(wq_t, 0, qb, fast=(qb == 0))
                    ktr[qb] = proj_rope_st(wk_t, 0, qb, fast=(qb == 0))
                qcols_t = qtr[qb]
                oA = psp.tile([128, 512], f32, tag="o", bufs=3)
                oB = psp.tile([128, 512], f32, tag="o", bufs=3)
                nkb = 4 * (qb + 1)
                for kb in range(nkb):
                    kt_t = ktr[kb // 4]
                    kcols = slice((kb % 4) * 128, (kb % 4) * 128 + 128)
                    jrel = kb - 4 * qb
                    lo = max(jrel, 0) * 128   # first valid q col in block
                    sub = slice(lo, 512)
                    stA = psp.tile([128, 512], f32, tag="st", bufs=3)
                    stB = psp.tile([128, 512], f32, tag="st", bufs=3)
                    nc.tensor.matmul(stA[:, sub], kt_t[0:64, kcols],
                                     qcols_t[0:64, sub])
                    nc.tensor.matmul(stB[:, sub], kt_t[64:128, kcols],
                                     qcols_t[64:128, sub])
                    pA = til([128, 512], bf16, "p", 8)
                    pB = til([128, 512], bf16, "p", 8)
                    nc.scalar.activation(pA[:, sub], stA[:, sub], AF.Exp,
                                         scale=SCALE)
                    nc.scalar.activation(pB[:, sub], stB[:, sub], AF.Exp,
                                         scale=SCALE)
                    if jrel >= 0:
                        tri = slice(lo, lo + 128)
                        nc.vector.tensor_mul(pA[:, tri], pA[:, tri], mask_t[:])
                        nc.vector.tensor_mul(pB[:, tri], pB[:, tri], mask_t[:])
                    nc.tensor.matmul(oA[:, sub], v_slice(kb, p, 0, 128),
                                     pA[:, sub],
                                     start=(kb == 0), stop=(kb == nkb - 1))
                    nc.tensor.matmul(oB[:, sub], v_slice(kb, p, 64, 192),
                                     pB[:, sub],
                                     start=(kb == 0), stop=(kb == nkb - 1))
                # normalize. A psum rows: [O_A | l_A]; B psum rows: [l_B | O_B]
                qcols = slice(qb * 512, (qb + 1) * 512)
                onrm = til([128, 512], bf16, "onrm", 4)
                rcA = til([128, 512], f32, "rcA", 2)
                nc.vector.reciprocal(rcA[64:128, :], oA[64:128, :])
                rcA2 = til([64, 512], f32, "rcA2", 3)
                nc.sync.dma_start(rcA2[:], rcA[64:128, :])
                nc.vector.tensor_mul(onrm[0:64, :], oA[0:64, :], rcA2[:])
                rcB = til([64, 512], f32, "rcB", 3)
                nc.vector.reciprocal(rcB[:], oB[0:64, :])
                rcB2 = til([128, 512], f32, "rcB2", 2)
                nc.sync.dma_start(rcB2[64:128, :], rcB[:])
                nc.vector.tensor_mul(onrm[64:128, :], oB[64:128, :],
                                     rcB2[64:128, :])
                nc.sync.dma_start(og_send[p][:, qcols].opt(), onrm[:])

        # ------------- per-pair exchange + output projection ---------------
        ofull = [None] * 8
        for p in range(4):
            if timing:
                nc.sync.dma_start(og_recv[p][0:128, :].opt(),
                                  og_send[p][:].opt())
                nc.sync.dma_start(og_recv[p][128:256, :].opt(),
                                  og_send[p][:].opt())
            else:
                nc.gpsimd.collective_compute(
                    "AllGather", mybir.AluOpType.bypass, replica_groups=GROUPS,
                    ins=[og_send[p][:].opt()], outs=[og_recv[p][:].opt()],
                )
            for g2 in range(2):
                t = til([128, S], bf16, "of" if p < 3 else "xt",
                        6 if p < 3 else 8)
                nc.sync.dma_start(
                    t[:], og_recv[p][g2 * 128:(g2 + 1) * 128, :].opt())
                ofull[4 * g2 + p] = t
        wt = load_w(woT)
        # two-pass accumulation: pass A (pairs 0-1) runs during the pair-2/3
        # attention, parking partials in DRAM (one tensor per st16 so the
        # passes pipeline); pass B adds pairs 2-3
        for st16 in range(16):
            ps = psp.tile([128, 512], f32, tag="proj", bufs=2)
            for i, dt in enumerate([0, 4, 1, 5]):
                nc.tensor.matmul(
                    ps[:],
                    ofull[dt][:, st16 * 128:(st16 + 1) * 128],
                    wt[dt][:],
                    start=(i == 0), stop=(i == 3),
                )
            a_sb = til([128, SL], f32, "osb", 4)
            nc.vector.tensor_copy(a_sb[:], ps[:])
            nc.sync.dma_start(acc_d[st16][:].opt(), a_sb[:])
        for st16 in range(16):
            a_rd = til([128, SL], f32, "ard", 8)
            nc.sync.dma_start(a_rd[:], acc_d[st16][:].opt())
            ps = psp.tile([128, 512], f32, tag="proj", bufs=2)
            for i, dt in enumerate([2, 6, 3, 7]):
                nc.tensor.matmul(
                    ps[:],
                    ofull[dt][:, st16 * 128:(st16 + 1) * 128],
                    wt[dt][:],
                    start=(i == 0), stop=(i == 3),
                )
            o_sb = til([128, SL], f32, "osb", 4)
            nc.vector.tensor_add(o_sb[:], ps[:], a_rd[:])
            nc.sync.dma_start(out[st16 * 128:(st16 + 1) * 128, :], o_sb[:])

def rope_perm_rows(heads):
    rows = []
    for h in heads:
        rows += [h * DH + j for j in range(0, DH, 2)]
        rows += [h * DH + j for j in range(1, DH, 2)]
    return np.array(rows)


def prep_inputs(x, WQ, WK, WV, WO, token_positions):
    x = np.asarray(x, np.float32)
    WQ = np.asarray(WQ, np.float32)
    WK = np.asarray(WK, np.float32)
    WV = np.asarray(WV, np.float32)
    WO = np.asarray(WO, np.float32)
    pos = np.asarray(token_positions).astype(np.float32)
    bf = ml_dtypes.bfloat16

    r = np.arange(128)
    invf = (10000.0 ** (-(r % 32) / 32.0)).astype(np.float32)
    sign = np.where((r % 64) < 32, -1.0, 1.0).astype(np.float32)
    ang = pos[None, :] * invf[:, None]
    cosr = np.cos(ang).astype(np.float32)
    sinr = np.sin(ang * sign[:, None]).astype(np.float32)

    in_maps = []
    for c in range(NCORE):
        b, g = divmod(c, 2)
        heads = list(range(8 * g, 8 * g + 8))
        perm = rope_perm_rows(heads)
        rows = slice(8 * g * DH, (8 * g + 8) * DH)
        in_maps.append({
            "xT": np.ascontiguousarray(x[b].T).astype(bf),
            "wqT": np.ascontiguousarray(WQ[perm, :].T).astype(bf),
            "wkT": np.ascontiguousarray(WK[perm, :].T).astype(bf),
            "wvT": np.ascontiguousarray(WV.T[:, rows]).astype(bf),
            "woT": np.ascontiguousarray(WO.T[:, g * SL:(g + 1) * SL]).astype(bf),
            "cosr": cosr,
            "sinr": sinr,
        })
    return in_maps


def assemble(results):
    B = NCORE // 2
    out = np.empty((B, S, D), np.float32)
    for b in range(B):
        out[b, :, 0:SL] = results[2 * b]["out"]
        out[b, :, SL:D] = results[2 * b + 1]["out"]
    return out


_NC = None


def _get_nc():
    global _NC
    if _NC is None:
        _NC = build()
    return _NC


def kernel(x, WQ, WK, WV, WO, token_positions):
    nc = _get_nc()
    in_maps = prep_inputs(x, WQ, WK, WV, WO, token_positions)
    res = run_bass_kernel_spmd(nc, in_maps, list(range(NCORE)))
    return assemble(res.results)



# revision 34
# speedup vs baseline: 1.0792x; 1.0792x over previous
"""Trainium2 Bass kernel: multi-head flash self-attention with RoPE.

Problem: x[4,2048,1024], 16 heads, dh=64, causal, RoPE(theta=10000), WO proj.

Sharding (8 cores): core c -> batch b=c//2, head-group g=c%2 (8 heads each).
Per core:
  - QKV projections of x[b] (bf16 matmuls, fp32 PSUM accumulation), inputs
    loaded with a handful of large rearranged-AP DMAs ordered by first use.
  - RoPE folded into a host-side weight-row permutation (per head: even dims
    then odd dims); the rotation's partition swap is done with cross-base DVE
    reads straight out of PSUM (no DMA shuffles).
  - Flash attention in S^T layout ([k,q] blocks). V is stored per (ktile,
    head-pair) as [V_A | ones | V_B] so each head's stationary operand is a
    contiguous 128 cols and the softmax denominators appear as 64 replicated
    PSUM rows. No max subtraction (scores ~ N(0,1) by construction). Scores
    for both heads of a pair land in one 2-bank PSUM tile so a single Exp
    activation covers them; the AV matmul for block kb-1 is interleaved after
    the scores of block kb to hide the exp/mask latency.
  - Pairwise AllGather of normalized O^T. Pair 3's exchange is chunked per
    512-query block and the output projection runs per chunk inside pair 3's
    loop, so the tail exposes only the last chunk.
Host reassembles: out[b] = concat(cols of core 2b, cols of core 2b+1).
"""
import sys

sys.path.insert(0, "/opt/trn_rl_repo")

import numpy as np
import ml_dtypes
import concourse.bass as bass
import concourse.bacc as bacc
import concourse.mybir as mybir
from concourse import tile
from concourse.bass_utils import run_bass_kernel_spmd

f32 = mybir.dt.float32
bf16 = mybir.dt.bfloat16
AF = mybir.ActivationFunctionType

S = 2048
D = 1024
H = 16
DH = 64
NCORE = 8
SL = 512           # local m dims (8 heads x 64)
SCALE = 1.0 / 8.0  # 1/sqrt(dh)
GROUPS = [[0, 1], [2, 3], [4, 5], [6, 7]]
VPP = 192          # v_store cols per (ktile, pair): [V_A | ones | V_B]
VKT = 4 * VPP      # v_store cols per ktile


def build(timing=False):
    nc = bacc.Bacc("TRN2", target_bir_lowering=False, debug=False,
                   num_devices=1 if timing else NCORE)

    xT = nc.dram_tensor("xT", [D, S], bf16, kind="ExternalInput").ap()
    wqT = nc.dram_tensor("wqT", [D, SL], bf16, kind="ExternalInput").ap()
    wkT = nc.dram_tensor("wkT", [D, SL], bf16, kind="ExternalInput").ap()
    wvT = nc.dram_tensor("wvT", [D, SL], bf16, kind="ExternalInput").ap()
    woT = nc.dram_tensor("woT", [D, SL], bf16, kind="ExternalInput").ap()
    permr = nc.dram_tensor("permr", [128, 128], bf16, kind="ExternalInput").ap()
    cosr = nc.dram_tensor("cosr", [128, S], f32, kind="ExternalInput").ap()
    sinr = nc.dram_tensor("sinr", [128, S], f32, kind="ExternalInput").ap()
    out = nc.dram_tensor("out", [S, SL], f32, kind="ExternalOutput").ap()

    og_send = [nc.dram_tensor(f"og_send{p}", [128, S], bf16) for p in range(3)]
    og_recv = [nc.dram_tensor(f"og_recv{p}", [256, S], bf16) for p in range(3)]
    og_send3 = [nc.dram_tensor(f"og_send3_{q}", [128, 512], bf16)
                for q in range(4)]
    og_recv3 = [nc.dram_tensor(f"og_recv3_{q}", [256, 512], bf16)
                for q in range(4)]

    with tile.TileContext(nc) as tc:
        _body(nc, tc, xT, wqT, wkT, wvT, woT, permr, cosr, sinr, out,
              og_send, og_recv, og_send3, og_recv3, timing)
    nc.compile()
    return nc


def _body(nc, tc, xT, wqT, wkT, wvT, woT, permr, cosr, sinr, out,
          og_send, og_recv, og_send3, og_recv3, timing=False):
    from contextlib import ExitStack
    ctx = ExitStack()
    with ctx:
        sb = ctx.enter_context(tc.tile_pool(name="sb", bufs=1))
        psp = ctx.enter_context(tc.tile_pool(name="psp", bufs=1, space="PSUM"))
        counter = [0]

        def til(shape, dtype, tag, bufs):
            counter[0] += 1
            return sb.tile(shape, dtype, tag=tag, bufs=bufs,
                           name=f"{tag}_{counter[0]}")

        # ---------------- consolidated input loads (SP queue) --------------
        # one big rearranged-AP DMA per tensor chunk, ordered by first use.
        def wload(wdram, cl, ch, tag=None, t=None):
            if t is None:
                t = til([128, 4096], bf16, tag, 1)
            dst = t[:].rearrange("p (dt c) -> p dt c", c=512)
            src = wdram.rearrange("(dt p) c -> p dt c", p=128)
            nc.sync.dma_start(dst[:, :, cl:ch], src[:, :, cl:ch])
            return t

        perm_t = til([128, 128], bf16, "perm", 1)
        nc.sync.dma_start(perm_t[:], permr[:])
        wq_t = wload(wqT, 0, 128, tag="wq")
        xt = til([128, 8 * S], bf16, "xt", 1)
        xt_r = xt[:].rearrange("p (dt c) -> p dt c", c=S)
        xT_r = xT.rearrange("(dt p) c -> p dt c", p=128)
        nc.sync.dma_start(xt_r[:, 0:4, 0:512], xT_r[:, 0:4, 0:512])
        nc.sync.dma_start(xt_r[:, 4:8, 0:512], xT_r[:, 4:8, 0:512])
        wk_t = wload(wkT, 0, 128, tag="wk")
        wv_t = wload(wvT, 0, 512, tag="wv")
        cos_t = til([128, S], f32, "cos", 1)
        nc.sync.dma_start(cos_t[:, 0:512], cosr[:, 0:512])
        sin_t = til([128, S], f32, "sin", 1)
        nc.sync.dma_start(sin_t[:, 0:512], sinr[:, 0:512])
        wload(wqT, 128, 512, t=wq_t)
        wload(wkT, 128, 512, t=wk_t)
        for c0 in range(512, S, 512):
            nc.sync.dma_start(xt_r[:, :, c0:c0 + 512],
                              xT_r[:, :, c0:c0 + 512])
        nc.sync.dma_start(cos_t[:, 512:S], cosr[:, 512:S])
        nc.sync.dma_start(sin_t[:, 512:S], sinr[:, 512:S])
        wo_t = wload(woT, 0, 512, tag="wo")

        def xs(dt, c0, c1):
            return xt[:, dt * S + c0:dt * S + c1]

        def ws(w, dt, c0, c1):
            return w[:, dt * 512 + c0:dt * 512 + c1]

        # 0/1 triangle mask [128,128]: 1 where c - r >= 0 (valid)
        mask_t = til([128, 128], bf16, "mask", 1)
        nc.gpsimd.memset(mask_t[:], 1.0)
        nc.gpsimd.affine_select(
            out=mask_t[:], in_=mask_t[:], compare_op=mybir.AluOpType.is_ge,
            fill=0.0, base=0, pattern=[[1, 128]], channel_multiplier=-1,
        )

        # v quarters: v_q[kt]; per pair p a VPP block: [V_A | ones | V_B].
        # only the `ones` columns need the memset.
        v_q = []
        for i in range(16):
            vq = til([128, VKT], bf16, "v", 16)
            vr = vq[:].rearrange("p (k o) -> p k o", o=VPP)
            nc.gpsimd.memset(vr[:, :, 64:128], 1.0)
            v_q.append(vq)

        def emit_v_quarter(i):
            for kt4 in range(4):
                kt = 4 * i + kt4
                ps = psp.tile([128, 512], f32, tag="o", bufs=2)
                for dt in range(8):
                    nc.tensor.matmul(
                        ps[:],
                        xs(dt, kt * 128, (kt + 1) * 128),
                        ws(wv_t, dt, 0, 512),
                        start=(dt == 0), stop=(dt == 7),
                    )
                vva = v_q[kt][:].rearrange("q (a c) -> q a c", c=64)
                psa = ps[:].rearrange("q (a c) -> q a c", c=64)
                nc.vector.tensor_copy(vva[:, 0:12:3, :], psa[:, 0:8:2, :])
                nc.vector.tensor_copy(vva[:, 2:12:3, :], psa[:, 1:8:2, :])

        def v_slice(kt, p, c0, c1):
            off = p * VPP
            return v_q[kt][:, off + c0:off + c1]

        # q-chain + k-chain of one (pair, st) fused into a [128, 1024] rope
        # pipeline: big = pre*cos + swap32(pre)*sin. The partition swap is a
        # PE matmul against a host-provided permutation matrix, so PSUM banks
        # free after one fast copy and the DVE tail is short and wait-free.
        # Emitted as two filler-sized phases (proj chains / perm+tail).
        def proj_phase1(mt, st, box):
            pre = til([128, 1024], bf16, "pre", 2)
            for half, wtiles in ((0, wq_t), (1, wk_t)):
                ps = psp.tile([128, 512], f32, tag="proj", bufs=2)
                for dt in range(8):
                    nc.tensor.matmul(
                        ps[:],
                        ws(wtiles, dt, mt * 128, (mt + 1) * 128),
                        xs(dt, st * 512, (st + 1) * 512),
                        start=(dt == 0), stop=(dt == 7),
                    )
                nc.vector.tensor_copy(pre[:, half * 512:(half + 1) * 512],
                                      ps[:])
            box.append(pre)

        def proj_phase2(mt, st, box):
            pre = box[0]
            big_t = til([128, 1024], bf16, "qk", 9)
            cols = slice(st * 512, (st + 1) * 512)
            tmp = til([128, 1024], f32, "tmp", 2)
            for half in (0, 1):
                hs = slice(half * 512, (half + 1) * 512)
                sw = psp.tile([128, 512], f32, tag="proj", bufs=2)
                nc.tensor.matmul(sw[:], perm_t[:], pre[:, hs],
                                 start=True, stop=True)
                nc.vector.tensor_mul(tmp[:, hs], pre[:, hs], cos_t[:, cols])
                sws = til([128, 512], f32, "swp", 2)
                nc.vector.tensor_mul(sws[:], sw[:], sin_t[:, cols])
                nc.vector.tensor_add(big_t[:, hs], tmp[:, hs], sws[:])
            return big_t

        qtr = [[None] * 4 for _ in range(4)]
        ktr = [[None] * 4 for _ in range(4)]
        ofull = [None] * 8  # [4*g2+p] -> [128, S] O^T bf16 (both groups)

        def attention(p, qb, fillers=()):
            qcols_t = qtr[p][qb]
            oA = psp.tile([128, 512], f32, tag="o", bufs=2)
            oB = psp.tile([128, 512], f32, tag="o", bufs=2)
            nkb = 4 * (qb + 1)
            pm = [None] * nkb
            subs = [None] * nkb

            def emit_scores(kb):
                kt_t = ktr[p][kb // 4]
                kcols = slice((kb % 4) * 128, (kb % 4) * 128 + 128)
                jrel = kb - 4 * qb
                lo = max(jrel, 0) * 128
                subs[kb] = slice(lo, 512)
                st = psp.tile([128, 1024], f32, tag="st", bufs=2)
                nc.tensor.matmul(st[:, lo:512], kt_t[0:64, kcols],
                                 qcols_t[0:64, lo:512])
                nc.tensor.matmul(st[:, 512 + lo:1024], kt_t[64:128, kcols],
                                 qcols_t[64:128, lo:512])
                pt = til([128, 1024], bf16, "p", 4)
                st3 = st[:].rearrange("q (two c) -> q two c", two=2)
                pt3 = pt[:].rearrange("q (two c) -> q two c", two=2)
                nc.scalar.activation(pt3[:, :, lo:512], st3[:, :, lo:512],
                                     AF.Exp, scale=SCALE)
                if jrel >= 0:
                    # masks on two engines so both P-tile writers finish fast
                    tri = slice(lo, lo + 128)
                    tri2 = slice(512 + lo, 512 + lo + 128)
                    nc.gpsimd.tensor_mul(pt[:, tri], pt[:, tri], mask_t[:])
                    nc.vector.tensor_mul(pt[:, tri2], pt[:, tri2], mask_t[:])
                pm[kb] = pt

            def emit_av(kb):
                lo = subs[kb].start
                sub = subs[kb]
                nc.tensor.matmul(oA[:, sub], v_slice(kb, p, 0, 128),
                                 pm[kb][:, lo:512],
                                 start=(kb == 0), stop=(kb == nkb - 1))
                nc.tensor.matmul(oB[:, sub], v_slice(kb, p, 64, 192),
                                 pm[kb][:, 512 + lo:1024],
                                 start=(kb == 0), stop=(kb == nkb - 1))

            # scores run 2 blocks ahead of AV so the exp/mask chain is off
            # the PE critical path; fillers (next-pair proj chains, output
            # projection chunks) are interleaved so the ACT engine's lower
            # per-block rate can catch up during them.
            fillers = list(fillers)
            nf = len(fillers)
            fpos = {}
            for i in range(nf):   # evenly over the kb in [2, nkb-1] loop
                kb_i = 2 + ((i + 1) * (nkb - 2) - 1) // (nf + 1)
                fpos.setdefault(min(kb_i, nkb - 1), []).append(fillers[i])
            emit_scores(0)
            emit_scores(1)
            for kb in range(2, nkb):
                emit_scores(kb)
                emit_av(kb - 2)
                for f in fpos.get(kb, ()):
                    f()
            emit_av(nkb - 2)
            emit_av(nkb - 1)

            # normalize. A psum rows: [O_A | l_A]; B psum rows: [l_B | O_B].
            # cross-base PSUM reads avoid any partition-move DMA.
            onrm = til([128, 512], bf16, "onrm", 4)
            rc = til([128, 512], f32, "rc", 2)
            nc.vector.reciprocal(rc[64:128, :], oA[64:128, :])
            nc.vector.tensor_mul(onrm[0:64, :], oA[0:64, :], rc[64:128, :])
            nc.vector.reciprocal(rc[0:64, :], oB[0:64, :])
            nc.vector.tensor_mul(onrm[64:128, :], oB[64:128, :], rc[0:64, :])
            qcols = slice(qb * 512, (qb + 1) * 512)
            if p < 3:
                nc.sync.dma_start(og_send[p][:, qcols].opt(), onrm[:])
            else:
                nc.sync.dma_start(og_send3[qb][:].opt(), onrm[:])

        def exchange(p):
            if timing:
                nc.sync.dma_start(og_recv[p][0:128, :].opt(),
                                  og_send[p][:].opt())
                nc.sync.dma_start(og_recv[p][128:256, :].opt(),
                                  og_send[p][:].opt())
            else:
                nc.gpsimd.collective_compute(
                    "AllGather", mybir.AluOpType.bypass, replica_groups=GROUPS,
                    ins=[og_send[p][:].opt()], outs=[og_recv[p][:].opt()],
                )
            for g2 in range(2):
                t = til([128, S], bf16, "of", 8)
                nc.sync.dma_start(
                    t[:], og_recv[p][g2 * 128:(g2 + 1) * 128, :].opt())
                ofull[4 * g2 + p] = t

        def exchange3(qb):
            if timing:
                nc.sync.dma_start(og_recv3[qb][0:128, :].opt(),
                                  og_send3[qb][:].opt())
                nc.sync.dma_start(og_recv3[qb][128:256, :].opt(),
                                  og_send3[qb][:].opt())
            else:
                nc.gpsimd.collective_compute(
                    "AllGather", mybir.AluOpType.bypass, replica_groups=GROUPS,
                    ins=[og_send3[qb][:].opt()], outs=[og_recv3[qb][:].opt()],
                )
            if ofull[3] is None:
                ofull[3] = til([128, S], bf16, "of", 8)
                ofull[7] = til([128, S], bf16, "of", 8)
            qcols = slice(qb * 512, (qb + 1) * 512)
            for g2 in range(2):
                nc.sync.dma_start(
                    ofull[4 * g2 + 3][:, qcols],
                    og_recv3[qb][g2 * 128:(g2 + 1) * 128, :].opt())

        def outproj_st16(st16):
            qcols = slice(st16 * 128, (st16 + 1) * 128)
            ps = psp.tile([128, 512], f32, tag="proj", bufs=2)
            for i, dt in enumerate([0, 4, 1, 5, 2, 6, 3, 7]):
                nc.tensor.matmul(ps[:], ofull[dt][:, qcols],
                                 ws(wo_t, dt, 0, 512),
                                 start=(i == 0), stop=(i == 7))
            o_sb = til([128, SL], f32, "osb", 4)
            nc.vector.tensor_copy(o_sb[:], ps[:])
            nc.sync.dma_start(out[qcols, :], o_sb[:])

        def outproj_chunk(qb):
            for st16 in range(4 * qb, 4 * qb + 4):
                outproj_st16(st16)

        # ---------------- main pair loop ----------------
        # pair p+1's Q/K projection is interleaved into pair p's qb loop,
        # BEFORE attention so its rope drain isn't queued behind the norm ops
        # on the DVE (which wait for attention to finish). The output
        # projection chunks are decoupled from pair 3's qb loop with enough
        # slack to hide the 3-hop exchange chain.
        def set_qk2(p1, st, box):
            big_t = proj_phase2(p1, st, box)
            qtr[p1][st] = big_t[:, 0:512]
            ktr[p1][st] = big_t[:, 512:1024]

        def proj_fillers(p1, st):
            box = []
            return [lambda: proj_phase1(p1, st, box),
                    lambda: set_qk2(p1, st, box)]

        def set_qk(p1, st):
            box = []
            proj_phase1(p1, st, box)
            set_qk2(p1, st, box)

        set_qk(0, 0)
        for p in range(4):
            for qb in range(4):
                fillers = []
                if p == 0:
                    emit_v_quarter(qb)
                    if qb < 3:
                        fillers = proj_fillers(0, qb + 1)
                if p < 3:
                    # next-pair proj chains ride one qb later so the short
                    # qb=0 window never carries chains, and each chain has a
                    # full attention block of drain slack.
                    if qb == 3:
                        fillers += proj_fillers(p + 1, 2) + proj_fillers(p + 1, 3)
                    elif qb > 0:
                        fillers += proj_fillers(p + 1, qb - 1)
                elif qb >= 2:
                    fillers = [(lambda s: lambda: outproj_st16(s))(st16)
                               for st16 in range(4 * (qb - 2), 4 * (qb - 2) + 4)]
                attention(p, qb, fillers)
                if p == 3:
                    exchange3(qb)
            if p < 3:
                exchange(p)
        outproj_chunk(2)
        outproj_chunk(3)


def rope_perm_rows(heads):
    rows = []
    for h in heads:
        rows += [h * DH + j for j in range(0, DH, 2)]
        rows += [h * DH + j for j in range(1, DH, 2)]
    return np.array(rows)


def prep_inputs(x, WQ, WK, WV, WO, token_positions):
    x = np.asarray(x, np.float32)
    WQ = np.asarray(WQ, np.float32)
    WK = np.asarray(WK, np.float32)
    WV = np.asarray(WV, np.float32)
    WO = np.asarray(WO, np.float32)
    pos = np.asarray(token_positions).astype(np.float32)
    bf = ml_dtypes.bfloat16

    permm = np.zeros((128, 128), np.float32)
    jj = np.arange(128)
    permm[jj, jj ^ 32] = 1.0

    r = np.arange(128)
    invf = (10000.0 ** (-(r % 32) / 32.0)).astype(np.float32)
    sign = np.where((r % 64) < 32, -1.0, 1.0).astype(np.float32)
    ang = pos[None, :] * invf[:, None]
    cosr = np.cos(ang).astype(np.float32)
    sinr = np.sin(ang * sign[:, None]).astype(np.float32)

    in_maps = []
    for c in range(NCORE):
        b, g = divmod(c, 2)
        heads = list(range(8 * g, 8 * g + 8))
        perm = rope_perm_rows(heads)
        rows = slice(8 * g * DH, (8 * g + 8) * DH)
        in_maps.append({
            "xT": np.ascontiguousarray(x[b].T).astype(bf),
            "wqT": np.ascontiguousarray(WQ[perm, :].T).astype(bf),
            "wkT": np.ascontiguousarray(WK[perm, :].T).astype(bf),
            "wvT": np.ascontiguousarray(WV.T[:, rows]).astype(bf),
            "woT": np.ascontiguousarray(WO.T[:, g * SL:(g + 1) * SL]).astype(bf),
            "permr": permm.astype(bf),
            "cosr": cosr,
            "sinr": sinr,
        })
    return in_maps


def assemble(results):
    B = NCORE // 2
    out = np.empty((B, S, D), np.float32)
    for b in range(B):
        out[b, :, 0:SL] = results[2 * b]["out"]
        out[b, :, SL:D] = results[2 * b + 1]["out"]
    return out


_NC = None


def _get_nc():
    global _NC
    if _NC is None:
        _NC = build()
    return _NC


def kernel(x, WQ, WK, WV, WO, token_positions):
    nc = _get_nc()
    in_maps = prep_inputs(x, WQ, WK, WV, WO, token_positions)
    res = run_bass_kernel_spmd(nc, in_maps, list(range(NCORE)))
    return assemble(res.results)


# revision 35
# speedup vs baseline: 1.1147x; 1.0329x over previous
"""Trainium2 Bass kernel: multi-head flash self-attention with RoPE.

Problem: x[4,2048,1024], 16 heads, dh=64, causal, RoPE(theta=10000), WO proj.

Sharding (8 cores): core c -> batch b=c//2, head-group g=c%2 (8 heads each).
Per core:
  - QKV projections of x[b] (bf16 matmuls, fp32 PSUM accumulation), inputs
    loaded with a handful of large rearranged-AP DMAs ordered by first use.
  - RoPE folded into a host-side weight-row permutation (per head: even dims
    then odd dims); the rotation's partition swap is done with cross-base DVE
    reads straight out of PSUM (no DMA shuffles).
  - Flash attention in S^T layout ([k,q] blocks). V is stored per (ktile,
    head-pair) as [V_A | ones | V_B] so each head's stationary operand is a
    contiguous 128 cols and the softmax denominators appear as 64 replicated
    PSUM rows. No max subtraction (scores ~ N(0,1) by construction). Scores
    for both heads of a pair land in one 2-bank PSUM tile so a single Exp
    activation covers them; the AV matmul for block kb-1 is interleaved after
    the scores of block kb to hide the exp/mask latency.
  - Pairwise AllGather of normalized O^T. Pair 3's exchange is chunked per
    512-query block and the output projection runs per chunk inside pair 3's
    loop, so the tail exposes only the last chunk.
Host reassembles: out[b] = concat(cols of core 2b, cols of core 2b+1).
"""
import sys

sys.path.insert(0, "/opt/trn_rl_repo")

import numpy as np
import ml_dtypes
import concourse.bass as bass
import concourse.bacc as bacc
import concourse.mybir as mybir
from concourse import tile
from concourse.bass_utils import run_bass_kernel_spmd

f32 = mybir.dt.float32
bf16 = mybir.dt.bfloat16
AF = mybir.ActivationFunctionType

S = 2048
D = 1024
H = 16
DH = 64
NCORE = 8
SL = 512           # local m dims (8 heads x 64)
SCALE = 1.0 / 8.0  # 1/sqrt(dh)
GROUPS = [[0, 1], [2, 3], [4, 5], [6, 7]]
VPP = 192          # v_store cols per (ktile, pair): [V_A | ones | V_B]
VKT = 4 * VPP      # v_store cols per ktile


def build(timing=False):
    nc = bacc.Bacc("TRN2", target_bir_lowering=False, debug=False,
                   num_devices=1 if timing else NCORE)

    xT = nc.dram_tensor("xT", [D, S], bf16, kind="ExternalInput").ap()
    wqT = nc.dram_tensor("wqT", [D, SL], bf16, kind="ExternalInput").ap()
    wkT = nc.dram_tensor("wkT", [D, SL], bf16, kind="ExternalInput").ap()
    wvT = nc.dram_tensor("wvT", [D, SL], bf16, kind="ExternalInput").ap()
    woT = nc.dram_tensor("woT", [D, SL], bf16, kind="ExternalInput").ap()
    permr = nc.dram_tensor("permr", [128, 128], bf16, kind="ExternalInput").ap()
    cosr = nc.dram_tensor("cosr", [128, S], f32, kind="ExternalInput").ap()
    sinr = nc.dram_tensor("sinr", [128, S], f32, kind="ExternalInput").ap()
    out = nc.dram_tensor("out", [S, SL], f32, kind="ExternalOutput").ap()

    og_send = [nc.dram_tensor(f"og_send{p}", [128, S], bf16) for p in range(3)]
    og_recv = [nc.dram_tensor(f"og_recv{p}", [256, S], bf16) for p in range(3)]
    og_send3 = [nc.dram_tensor(f"og_send3_{q}", [128, 512], bf16)
                for q in range(4)]
    og_recv3 = [nc.dram_tensor(f"og_recv3_{q}", [256, 512], bf16)
                for q in range(4)]

    with tile.TileContext(nc) as tc:
        _body(nc, tc, xT, wqT, wkT, wvT, woT, permr, cosr, sinr, out,
              og_send, og_recv, og_send3, og_recv3, timing)
    nc.compile()
    return nc


def _body(nc, tc, xT, wqT, wkT, wvT, woT, permr, cosr, sinr, out,
          og_send, og_recv, og_send3, og_recv3, timing=False):
    from contextlib import ExitStack
    ctx = ExitStack()
    with ctx:
        sb = ctx.enter_context(tc.tile_pool(name="sb", bufs=1))
        psp = ctx.enter_context(tc.tile_pool(name="psp", bufs=1, space="PSUM"))
        counter = [0]

        def til(shape, dtype, tag, bufs):
            counter[0] += 1
            return sb.tile(shape, dtype, tag=tag, bufs=bufs,
                           name=f"{tag}_{counter[0]}")

        # ---------------- consolidated input loads (SP queue) --------------
        # one big rearranged-AP DMA per tensor chunk, ordered by first use.
        def wload(wdram, cl, ch, tag=None, t=None):
            if t is None:
                t = til([128, 4096], bf16, tag, 1)
            dst = t[:].rearrange("p (dt c) -> p dt c", c=512)
            src = wdram.rearrange("(dt p) c -> p dt c", p=128)
            nc.sync.dma_start(dst[:, :, cl:ch], src[:, :, cl:ch])
            return t

        perm_t = til([128, 128], bf16, "perm", 1)
        nc.sync.dma_start(perm_t[:], permr[:])
        wq_t = wload(wqT, 0, 128, tag="wq")
        xt = til([128, 8 * S], bf16, "xt", 1)
        xt_r = xt[:].rearrange("p (dt c) -> p dt c", c=S)
        xT_r = xT.rearrange("(dt p) c -> p dt c", p=128)
        nc.sync.dma_start(xt_r[:, 0:4, 0:512], xT_r[:, 0:4, 0:512])
        nc.sync.dma_start(xt_r[:, 4:8, 0:512], xT_r[:, 4:8, 0:512])
        wk_t = wload(wkT, 0, 128, tag="wk")
        wv_t = wload(wvT, 0, 512, tag="wv")
        cos_t = til([128, S], f32, "cos", 1)
        nc.sync.dma_start(cos_t[:, 0:512], cosr[:, 0:512])
        sin_t = til([128, S], f32, "sin", 1)
        nc.sync.dma_start(sin_t[:, 0:512], sinr[:, 0:512])
        wload(wqT, 128, 512, t=wq_t)
        wload(wkT, 128, 512, t=wk_t)
        for c0 in range(512, S, 512):
            nc.sync.dma_start(xt_r[:, :, c0:c0 + 512],
                              xT_r[:, :, c0:c0 + 512])
        nc.sync.dma_start(cos_t[:, 512:S], cosr[:, 512:S])
        nc.sync.dma_start(sin_t[:, 512:S], sinr[:, 512:S])
        wo_t = wload(woT, 0, 512, tag="wo")

        def xs(dt, c0, c1):
            return xt[:, dt * S + c0:dt * S + c1]

        def ws(w, dt, c0, c1):
            return w[:, dt * 512 + c0:dt * 512 + c1]

        # 0/1 triangle mask [128,128]: 1 where c - r >= 0 (valid)
        mask_t = til([128, 128], bf16, "mask", 1)
        nc.gpsimd.memset(mask_t[:], 1.0)
        nc.gpsimd.affine_select(
            out=mask_t[:], in_=mask_t[:], compare_op=mybir.AluOpType.is_ge,
            fill=0.0, base=0, pattern=[[1, 128]], channel_multiplier=-1,
        )

        # v quarters: v_q[kt]; per pair p a VPP block: [V_A | ones | V_B].
        # only the `ones` columns need the memset.
        v_q = []
        for i in range(16):
            vq = til([128, VKT], bf16, "v", 16)
            vr = vq[:].rearrange("p (k o) -> p k o", o=VPP)
            nc.gpsimd.memset(vr[:, :, 64:128], 1.0)
            v_q.append(vq)

        def emit_v_quarter(i):
            for kt4 in (3, 2, 1, 0):
                kt = 4 * i + kt4
                ps = psp.tile([128, 512], f32, tag="o", bufs=2)
                for dt in range(8):
                    nc.tensor.matmul(
                        ps[:],
                        xs(dt, kt * 128, (kt + 1) * 128),
                        ws(wv_t, dt, 0, 512),
                        start=(dt == 0), stop=(dt == 7),
                    )
                vva = v_q[kt][:].rearrange("q (a c) -> q a c", c=64)
                psa = ps[:].rearrange("q (a c) -> q a c", c=64)
                nc.vector.tensor_copy(vva[:, 0:12:3, :], psa[:, 0:8:2, :])
                nc.vector.tensor_copy(vva[:, 2:12:3, :], psa[:, 1:8:2, :])

        def v_slice(kt, p, c0, c1):
            off = p * VPP
            return v_q[kt][:, off + c0:off + c1]

        # q-chain + k-chain of one (pair, st) fused into a [128, 1024] rope
        # pipeline: big = pre*cos + swap32(pre)*sin. The partition swap is a
        # PE matmul against a host-provided permutation matrix, so PSUM banks
        # free after one fast copy and the DVE tail is short and wait-free.
        # Emitted as two filler-sized phases (proj chains / perm+tail).
        def proj_phase1(mt, st, box):
            pre = til([128, 1024], bf16, "pre", 2)
            for half, wtiles in ((0, wq_t), (1, wk_t)):
                ps = psp.tile([128, 512], f32, tag="proj", bufs=2)
                for dt in range(8):
                    nc.tensor.matmul(
                        ps[:],
                        ws(wtiles, dt, mt * 128, (mt + 1) * 128),
                        xs(dt, st * 512, (st + 1) * 512),
                        start=(dt == 0), stop=(dt == 7),
                    )
                nc.vector.tensor_copy(pre[:, half * 512:(half + 1) * 512],
                                      ps[:])
            box.append(pre)

        def proj_phase2(mt, st, box):
            pre = box[0]
            big_t = til([128, 1024], bf16, "qk", 9)
            cols = slice(st * 512, (st + 1) * 512)
            tmp = til([128, 1024], f32, "tmp", 2)
            for half in (0, 1):
                hs = slice(half * 512, (half + 1) * 512)
                sw = psp.tile([128, 512], f32, tag="proj", bufs=2)
                nc.tensor.matmul(sw[:], perm_t[:], pre[:, hs],
                                 start=True, stop=True)
                nc.vector.tensor_mul(tmp[:, hs], pre[:, hs], cos_t[:, cols])
                sws = til([128, 512], f32, "swp", 2)
                nc.vector.tensor_mul(sws[:], sw[:], sin_t[:, cols])
                nc.vector.tensor_add(big_t[:, hs], tmp[:, hs], sws[:])
            return big_t

        qtr = [[None] * 4 for _ in range(4)]
        ktr = [[None] * 4 for _ in range(4)]
        ofull = [None] * 8  # [4*g2+p] -> [128, S] O^T bf16 (both groups)

        def attention(p, qb, fillers=()):
            qcols_t = qtr[p][qb]
            oA = psp.tile([128, 512], f32, tag="o", bufs=2)
            oB = psp.tile([128, 512], f32, tag="o", bufs=2)
            nkb = 4 * (qb + 1)
            pm = [None] * nkb
            subs = [None] * nkb

            def emit_scores(kb):
                kt_t = ktr[p][kb // 4]
                kcols = slice((kb % 4) * 128, (kb % 4) * 128 + 128)
                jrel = kb - 4 * qb
                lo = max(jrel, 0) * 128
                subs[kb] = slice(lo, 512)
                st = psp.tile([128, 1024], f32, tag="st", bufs=2)
                nc.tensor.matmul(st[:, lo:512], kt_t[0:64, kcols],
                                 qcols_t[0:64, lo:512])
                nc.tensor.matmul(st[:, 512 + lo:1024], kt_t[64:128, kcols],
                                 qcols_t[64:128, lo:512])
                pt = til([128, 1024], bf16, "p", 4)
                st3 = st[:].rearrange("q (two c) -> q two c", two=2)
                pt3 = pt[:].rearrange("q (two c) -> q two c", two=2)
                nc.scalar.activation(pt3[:, :, lo:512], st3[:, :, lo:512],
                                     AF.Exp, scale=SCALE)
                if jrel >= 0:
                    # masks on two engines so both P-tile writers finish fast
                    tri = slice(lo, lo + 128)
                    tri2 = slice(512 + lo, 512 + lo + 128)
                    nc.gpsimd.tensor_mul(pt[:, tri], pt[:, tri], mask_t[:])
                    nc.vector.tensor_mul(pt[:, tri2], pt[:, tri2], mask_t[:])
                pm[kb] = pt

            def emit_av(kb, first, last):
                lo = subs[kb].start
                sub = subs[kb]
                nc.tensor.matmul(oA[:, sub], v_slice(kb, p, 0, 128),
                                 pm[kb][:, lo:512],
                                 start=first, stop=last)
                nc.tensor.matmul(oB[:, sub], v_slice(kb, p, 64, 192),
                                 pm[kb][:, 512 + lo:1024],
                                 start=first, stop=last)

            # scores run 2 blocks ahead of AV so the exp/mask chain is off
            # the PE critical path; the diagonal (smallest) blocks go first
            # so the qb-boundary recycle ladder is short; fillers (next-pair
            # proj chains, output projection chunks) are interleaved to give
            # the other engines catch-up room.
            kbs = [4 * qb + 3, 4 * qb + 2, 4 * qb + 1, 4 * qb] + \
                list(range(0, 4 * qb))
            fillers = list(fillers)
            nf = len(fillers)
            fpos = {}
            for i in range(nf):   # evenly over positions [2, nkb-1]
                p_i = 2 + ((i + 1) * (nkb - 2) - 1) // (nf + 1)
                fpos.setdefault(min(p_i, nkb - 1), []).append(fillers[i])
            emit_scores(kbs[0])
            emit_scores(kbs[1])
            for j in range(2, nkb):
                emit_scores(kbs[j])
                emit_av(kbs[j - 2], j - 2 == 0, False)
                for f in fpos.get(j, ()):
                    f()
            emit_av(kbs[nkb - 2], False, False)
            emit_av(kbs[nkb - 1], False, True)

            # normalize. A psum rows: [O_A | l_A]; B psum rows: [l_B | O_B].
            # cross-base PSUM reads avoid any partition-move DMA.
            onrm = til([128, 512], bf16, "onrm", 4)
            rc = til([128, 512], f32, "rc", 2)
            nc.vector.reciprocal(rc[64:128, :], oA[64:128, :])
            nc.vector.tensor_mul(onrm[0:64, :], oA[0:64, :], rc[64:128, :])
            nc.vector.reciprocal(rc[0:64, :], oB[0:64, :])
            nc.vector.tensor_mul(onrm[64:128, :], oB[64:128, :], rc[0:64, :])
            qcols = slice(qb * 512, (qb + 1) * 512)
            if p < 3:
                nc.sync.dma_start(og_send[p][:, qcols].opt(), onrm[:])
            else:
                nc.sync.dma_start(og_send3[qb][:].opt(), onrm[:])

        def exchange(p):
            if timing:
                nc.sync.dma_start(og_recv[p][0:128, :].opt(),
                                  og_send[p][:].opt())
                nc.sync.dma_start(og_recv[p][128:256, :].opt(),
                                  og_send[p][:].opt())
            else:
                nc.gpsimd.collective_compute(
                    "AllGather", mybir.AluOpType.bypass, replica_groups=GROUPS,
                    ins=[og_send[p][:].opt()], outs=[og_recv[p][:].opt()],
                )
            for g2 in range(2):
                t = til([128, S], bf16, "of", 8)
                nc.sync.dma_start(
                    t[:], og_recv[p][g2 * 128:(g2 + 1) * 128, :].opt())
                ofull[4 * g2 + p] = t

        def exchange3(qb):
            if timing:
                nc.sync.dma_start(og_recv3[qb][0:128, :].opt(),
                                  og_send3[qb][:].opt())
                nc.sync.dma_start(og_recv3[qb][128:256, :].opt(),
                                  og_send3[qb][:].opt())
            else:
                nc.gpsimd.collective_compute(
                    "AllGather", mybir.AluOpType.bypass, replica_groups=GROUPS,
                    ins=[og_send3[qb][:].opt()], outs=[og_recv3[qb][:].opt()],
                )
            if ofull[3] is None:
                ofull[3] = til([128, S], bf16, "of", 8)
                ofull[7] = til([128, S], bf16, "of", 8)
            qcols = slice(qb * 512, (qb + 1) * 512)
            for g2 in range(2):
                nc.sync.dma_start(
                    ofull[4 * g2 + 3][:, qcols],
                    og_recv3[qb][g2 * 128:(g2 + 1) * 128, :].opt())

        def outproj_st16(st16):
            qcols = slice(st16 * 128, (st16 + 1) * 128)
            ps = psp.tile([128, 512], f32, tag="proj", bufs=2)
            for i, dt in enumerate([0, 4, 1, 5, 2, 6, 3, 7]):
                nc.tensor.matmul(ps[:], ofull[dt][:, qcols],
                                 ws(wo_t, dt, 0, 512),
                                 start=(i == 0), stop=(i == 7))
            o_sb = til([128, SL], f32, "osb", 4)
            nc.vector.tensor_copy(o_sb[:], ps[:])
            nc.sync.dma_start(out[qcols, :], o_sb[:])

        def outproj_chunk(qb):
            for st16 in range(4 * qb, 4 * qb + 4):
                outproj_st16(st16)

        # ---------------- main pair loop ----------------
        # pair p+1's Q/K projection is interleaved into pair p's qb loop,
        # BEFORE attention so its rope drain isn't queued behind the norm ops
        # on the DVE (which wait for attention to finish). The output
        # projection chunks are decoupled from pair 3's qb loop with enough
        # slack to hide the 3-hop exchange chain.
        def set_qk2(p1, st, box):
            big_t = proj_phase2(p1, st, box)
            qtr[p1][st] = big_t[:, 0:512]
            ktr[p1][st] = big_t[:, 512:1024]

        def proj_fillers(p1, st):
            box = []
            return [lambda: proj_phase1(p1, st, box),
                    lambda: set_qk2(p1, st, box)]

        def set_qk(p1, st):
            box = []
            proj_phase1(p1, st, box)
            set_qk2(p1, st, box)

        set_qk(0, 0)
        for p in range(4):
            for qb in range(4):
                fillers = []
                if p == 0:
                    emit_v_quarter(qb)
                    if qb < 3:
                        fillers = proj_fillers(0, qb + 1)
                if p < 3:
                    # next-pair proj chains ride one qb later so the short
                    # qb=0 window never carries chains, and each chain has a
                    # full attention block of drain slack.
                    if qb == 3:
                        fillers += proj_fillers(p + 1, 2) + proj_fillers(p + 1, 3)
                    elif qb > 0:
                        fillers += proj_fillers(p + 1, qb - 1)
                elif qb >= 2:
                    fillers = [(lambda s: lambda: outproj_st16(s))(st16)
                               for st16 in range(4 * (qb - 2), 4 * (qb - 2) + 4)]
                attention(p, qb, fillers)
                if p == 3:
                    exchange3(qb)
            if p < 3:
                exchange(p)
        outproj_chunk(2)
        outproj_chunk(3)


def rope_perm_rows(heads):
    rows = []
    for h in heads:
        rows += [h * DH + j for j in range(0, DH, 2)]
        rows += [h * DH + j for j in range(1, DH, 2)]
    return np.array(rows)


def prep_inputs(x, WQ, WK, WV, WO, token_positions):
    x = np.asarray(x, np.float32)
    WQ = np.asarray(WQ, np.float32)
    WK = np.asarray(WK, np.float32)
    WV = np.asarray(WV, np.float32)
    WO = np.asarray(WO, np.float32)
    pos = np.asarray(token_positions).astype(np.float32)
    bf = ml_dtypes.bfloat16

    permm = np.zeros((128, 128), np.float32)
    jj = np.arange(128)
    permm[jj, jj ^ 32] = 1.0

    r = np.arange(128)
    invf = (10000.0 ** (-(r % 32) / 32.0)).astype(np.float32)
    sign = np.where((r % 64) < 32, -1.0, 1.0).astype(np.float32)
    ang = pos[None, :] * invf[:, None]
    cosr = np.cos(ang).astype(np.float32)
    sinr = np.sin(ang * sign[:, None]).astype(np.float32)

    in_maps = []
    for c in range(NCORE):
        b, g = divmod(c, 2)
        heads = list(range(8 * g, 8 * g + 8))
        perm = rope_perm_rows(heads)
        rows = slice(8 * g * DH, (8 * g + 8) * DH)
        in_maps.append({
            "xT": np.ascontiguousarray(x[b].T).astype(bf),
            "wqT": np.ascontiguousarray(WQ[perm, :].T).astype(bf),
            "wkT": np.ascontiguousarray(WK[perm, :].T).astype(bf),
            "wvT": np.ascontiguousarray(WV.T[:, rows]).astype(bf),
            "woT": np.ascontiguousarray(WO.T[:, g * SL:(g + 1) * SL]).astype(bf),
            "permr": permm.astype(bf),
            "cosr": cosr,
            "sinr": sinr,
        })
    return in_maps


def assemble(results):
    B = NCORE // 2
    out = np.empty((B, S, D), np.float32)
    for b in range(B):
        out[b, :, 0:SL] = results[2 * b]["out"]
        out[b, :, SL:D] = results[2 * b + 1]["out"]
    return out


_NC = None


def _get_nc():
    global _NC
    if _NC is None:
        _NC = build()
    return _NC


def kernel(x, WQ, WK, WV, WO, token_positions):
    nc = _get_nc()
    in_maps = prep_inputs(x, WQ, WK, WV, WO, token_positions)
    res = run_bass_kernel_spmd(nc, in_maps, list(range(NCORE)))
    return assemble(res.results)


# revision 51
# speedup vs baseline: 1.1197x; 1.0044x over previous
"""Trainium2 Bass kernel: multi-head flash self-attention with RoPE.

Problem: x[4,2048,1024], 16 heads, dh=64, causal, RoPE(theta=10000), WO proj.

Sharding (8 cores): core c -> batch b=c//2, head-group g=c%2 (8 heads each).
Per core:
  - QKV projections of x[b] (bf16 matmuls, fp32 PSUM accumulation), inputs
    loaded with a handful of large rearranged-AP DMAs ordered by first use.
  - RoPE folded into a host-side weight-row permutation (per head: even dims
    then odd dims); the rotation's partition swap is done with cross-base DVE
    reads straight out of PSUM (no DMA shuffles).
  - Flash attention in S^T layout ([k,q] blocks). V is stored per (ktile,
    head-pair) as [V_A | ones | V_B] so each head's stationary operand is a
    contiguous 128 cols and the softmax denominators appear as 64 replicated
    PSUM rows. No max subtraction (scores ~ N(0,1) by construction). Scores
    for both heads of a pair land in one 2-bank PSUM tile so a single Exp
    activation covers them; the AV matmul for block kb-1 is interleaved after
    the scores of block kb to hide the exp/mask latency.
  - Pairwise AllGather of normalized O^T. Pair 3's exchange is chunked per
    512-query block and the output projection runs per chunk inside pair 3's
    loop, so the tail exposes only the last chunk.
Host reassembles: out[b] = concat(cols of core 2b, cols of core 2b+1).
"""
import sys

sys.path.insert(0, "/opt/trn_rl_repo")

import numpy as np
import ml_dtypes
import concourse.bass as bass
import concourse.bacc as bacc
import concourse.mybir as mybir
from concourse import tile
from concourse.bass_utils import run_bass_kernel_spmd

f32 = mybir.dt.float32
bf16 = mybir.dt.bfloat16
AF = mybir.ActivationFunctionType

S = 2048
D = 1024
H = 16
DH = 64
NCORE = 8
SL = 512           # local m dims (8 heads x 64)
SCALE = 1.0 / 8.0  # 1/sqrt(dh)
GROUPS = [[0, 1], [2, 3], [4, 5], [6, 7]]
VPP = 192          # v_store cols per (ktile, pair): [V_A | ones | V_B]
VKT = 4 * VPP      # v_store cols per ktile


def build(timing=False):
    nc = bacc.Bacc("TRN2", target_bir_lowering=False, debug=False,
                   num_devices=1 if timing else NCORE)

    xT = nc.dram_tensor("xT", [D, S], bf16, kind="ExternalInput").ap()
    wqT = nc.dram_tensor("wqT", [D, SL], bf16, kind="ExternalInput").ap()
    wkT = nc.dram_tensor("wkT", [D, SL], bf16, kind="ExternalInput").ap()
    wvT = nc.dram_tensor("wvT", [D, SL], bf16, kind="ExternalInput").ap()
    woT = nc.dram_tensor("woT", [D, SL], bf16, kind="ExternalInput").ap()
    permr = nc.dram_tensor("permr", [128, 128], bf16, kind="ExternalInput").ap()
    cosr = nc.dram_tensor("cosr", [128, S], f32, kind="ExternalInput").ap()
    sinr = nc.dram_tensor("sinr", [128, S], f32, kind="ExternalInput").ap()
    out = nc.dram_tensor("out", [S, SL], f32, kind="ExternalOutput").ap()

    og_send = [nc.dram_tensor(f"og_send{p}", [128, S], bf16) for p in range(3)]
    og_recv = [nc.dram_tensor(f"og_recv{p}", [256, S], bf16) for p in range(3)]
    og_send3 = [nc.dram_tensor(f"og_send3_{q}", [128, 512], bf16)
                for q in range(4)]
    og_recv3 = [nc.dram_tensor(f"og_recv3_{q}", [256, 512], bf16)
                for q in range(4)]

    with tile.TileContext(nc) as tc:
        _body(nc, tc, xT, wqT, wkT, wvT, woT, permr, cosr, sinr, out,
              og_send, og_recv, og_send3, og_recv3, timing)
    nc.compile()
    return nc


def _body(nc, tc, xT, wqT, wkT, wvT, woT, permr, cosr, sinr, out,
          og_send, og_recv, og_send3, og_recv3, timing=False):
    from contextlib import ExitStack
    ctx = ExitStack()
    with ctx:
        sb = ctx.enter_context(tc.tile_pool(name="sb", bufs=1))
        psp = ctx.enter_context(tc.tile_pool(name="psp", bufs=1, space="PSUM"))
        counter = [0]

        def til(shape, dtype, tag, bufs):
            counter[0] += 1
            return sb.tile(shape, dtype, tag=tag, bufs=bufs,
                           name=f"{tag}_{counter[0]}")

        # ---------------- consolidated input loads (SP queue) --------------
        # one big rearranged-AP DMA per tensor chunk, ordered by first use.
        def wload(wdram, cl, ch, tag=None, t=None):
            if t is None:
                t = til([128, 4096], bf16, tag, 1)
            dst = t[:].rearrange("p (dt c) -> p dt c", c=512)
            src = wdram.rearrange("(dt p) c -> p dt c", p=128)
            nc.sync.dma_start(dst[:, :, cl:ch], src[:, :, cl:ch])
            return t

        perm_t = til([128, 128], bf16, "perm", 1)
        nc.sync.dma_start(perm_t[:], permr[:])
        wq_t = wload(wqT, 0, 128, tag="wq")
        xt = til([128, 8 * S], bf16, "xt", 1)
        xt_r = xt[:].rearrange("p (dt c) -> p dt c", c=S)
        xT_r = xT.rearrange("(dt p) c -> p dt c", p=128)
        nc.sync.dma_start(xt_r[:, 0:4, 0:512], xT_r[:, 0:4, 0:512])
        nc.sync.dma_start(xt_r[:, 4:8, 0:512], xT_r[:, 4:8, 0:512])
        wk_t = wload(wkT, 0, 128, tag="wk")
        wv_t = wload(wvT, 0, 512, tag="wv")
        cos_t = til([128, S], f32, "cos", 1)
        nc.sync.dma_start(cos_t[:, 0:512], cosr[:, 0:512])
        sin_t = til([128, S], f32, "sin", 1)
        nc.sync.dma_start(sin_t[:, 0:512], sinr[:, 0:512])
        wload(wqT, 128, 512, t=wq_t)
        wload(wkT, 128, 512, t=wk_t)
        for c0 in range(512, S, 512):
            nc.sync.dma_start(xt_r[:, :, c0:c0 + 512],
                              xT_r[:, :, c0:c0 + 512])
        nc.sync.dma_start(cos_t[:, 512:S], cosr[:, 512:S])
        nc.sync.dma_start(sin_t[:, 512:S], sinr[:, 512:S])
        wo_t = wload(woT, 0, 512, tag="wo")

        def xs(dt, c0, c1):
            return xt[:, dt * S + c0:dt * S + c1]

        def ws(w, dt, c0, c1):
            return w[:, dt * 512 + c0:dt * 512 + c1]

        # 0/1 triangle mask [128,128]: 1 where c - r >= 0 (valid)
        mask_t = til([128, 128], bf16, "mask", 1)
        nc.gpsimd.memset(mask_t[:], 1.0)
        nc.gpsimd.affine_select(
            out=mask_t[:], in_=mask_t[:], compare_op=mybir.AluOpType.is_ge,
            fill=0.0, base=0, pattern=[[1, 128]], channel_multiplier=-1,
        )

        # v quarters: v_q[kt]; per pair p a VPP block: [V_A | ones | V_B].
        # only the `ones` columns need the memset.
        v_q = []
        for i in range(16):
            vq = til([128, VKT], bf16, "v", 16)
            vr = vq[:].rearrange("p (k o) -> p k o", o=VPP)
            nc.gpsimd.memset(vr[:, :, 64:128], 1.0)
            v_q.append(vq)

        def emit_v_quarter(i):
            for kt4 in (3, 2, 1, 0):
                kt = 4 * i + kt4
                ps = psp.tile([128, 512], f32, tag="o", bufs=2)
                for dt in range(8):
                    nc.tensor.matmul(
                        ps[:],
                        xs(dt, kt * 128, (kt + 1) * 128),
                        ws(wv_t, dt, 0, 512),
                        start=(dt == 0), stop=(dt == 7),
                    )
                vva = v_q[kt][:].rearrange("q (a c) -> q a c", c=64)
                psa = ps[:].rearrange("q (a c) -> q a c", c=64)
                nc.vector.tensor_copy(vva[:, 0:12:3, :], psa[:, 0:8:2, :])
                nc.vector.tensor_copy(vva[:, 2:12:3, :], psa[:, 1:8:2, :])

        def v_slice(kt, p, c0, c1):
            off = p * VPP
            return v_q[kt][:, off + c0:off + c1]

        # q-chain + k-chain of one (pair, st) fused into a [128, 1024] rope
        # pipeline: big = pre*cos + swap32(pre)*sin. The partition swap is a
        # PE matmul against a host-provided permutation matrix, so PSUM banks
        # free after one fast copy and the DVE tail is short and wait-free.
        # Emitted as two filler-sized phases (proj chains / perm+tail).
        def proj_phase1(mt, st, box):
            pre = til([128, 1024], bf16, "pre", 2)
            for half, wtiles in ((0, wq_t), (1, wk_t)):
                ps = psp.tile([128, 512], f32, tag="proj", bufs=2)
                for dt in range(8):
                    nc.tensor.matmul(
                        ps[:],
                        ws(wtiles, dt, mt * 128, (mt + 1) * 128),
                        xs(dt, st * 512, (st + 1) * 512),
                        start=(dt == 0), stop=(dt == 7),
                    )
                nc.vector.tensor_copy(pre[:, half * 512:(half + 1) * 512],
                                      ps[:])
            box.append(pre)

        def proj_phase2(mt, st, box):
            pre = box[0]
            big_t = til([128, 1024], bf16, "qk", 9)
            cols = slice(st * 512, (st + 1) * 512)
            tmp = til([128, 1024], f32, "tmp", 2)
            for half in (0, 1):
                hs = slice(half * 512, (half + 1) * 512)
                sw = psp.tile([128, 512], f32, tag="proj", bufs=2)
                nc.tensor.matmul(sw[:], perm_t[:], pre[:, hs],
                                 start=True, stop=True)
                nc.vector.tensor_mul(tmp[:, hs], pre[:, hs], cos_t[:, cols])
                sws = til([128, 512], f32, "swp", 3)
                nc.vector.tensor_mul(sws[:], sw[:], sin_t[:, cols])
                nc.vector.tensor_add(big_t[:, hs], tmp[:, hs], sws[:])
            return big_t

        qtr = [[None] * 4 for _ in range(4)]
        ktr = [[None] * 4 for _ in range(4)]
        ofull = [None] * 8  # [4*g2+p] -> [128, S] O^T bf16 (both groups)

        def attention(p, qb, fillers=()):
            qcols_t = qtr[p][qb]
            oA = psp.tile([128, 512], f32, tag="o", bufs=2)
            oB = psp.tile([128, 512], f32, tag="o", bufs=2)
            nkb = 4 * (qb + 1)
            pm = [None] * nkb
            subs = [None] * nkb

            def emit_scores(kb):
                kt_t = ktr[p][kb // 4]
                kcols = slice((kb % 4) * 128, (kb % 4) * 128 + 128)
                jrel = kb - 4 * qb
                lo = max(jrel, 0) * 128
                subs[kb] = slice(lo, 512)
                st = psp.tile([128, 1024], f32, tag="st", bufs=2)
                nc.tensor.matmul(st[:, lo:512], kt_t[0:64, kcols],
                                 qcols_t[0:64, lo:512])
                nc.tensor.matmul(st[:, 512 + lo:1024], kt_t[64:128, kcols],
                                 qcols_t[64:128, lo:512])
                pt = til([128, 1024], bf16, "p", 6)
                st3 = st[:].rearrange("q (two c) -> q two c", two=2)
                pt3 = pt[:].rearrange("q (two c) -> q two c", two=2)
                nc.scalar.activation(pt3[:, :, lo:512], st3[:, :, lo:512],
                                     AF.Exp, scale=SCALE)
                if jrel >= 0:
                    # masks on two engines so both P-tile writers finish fast
                    tri = slice(lo, lo + 128)
                    tri2 = slice(512 + lo, 512 + lo + 128)
                    nc.gpsimd.tensor_mul(pt[:, tri], pt[:, tri], mask_t[:])
                    nc.vector.tensor_mul(pt[:, tri2], pt[:, tri2], mask_t[:])
                pm[kb] = pt

            def emit_av(kb, first, last):
                lo = subs[kb].start
                sub = subs[kb]
                nc.tensor.matmul(oA[:, sub], v_slice(kb, p, 0, 128),
                                 pm[kb][:, lo:512],
                                 start=first, stop=last)
                nc.tensor.matmul(oB[:, sub], v_slice(kb, p, 64, 192),
                                 pm[kb][:, 512 + lo:1024],
                                 start=first, stop=last)

            # scores run 2 blocks ahead of AV so the exp/mask chain is off
            # the PE critical path; the diagonal (smallest) blocks go first
            # so the qb-boundary recycle ladder is short; fillers (next-pair
            # proj chains, output projection chunks) are interleaved to give
            # the other engines catch-up room.
            kbs = [4 * qb + 3, 4 * qb + 2, 4 * qb + 1, 4 * qb] + \
                list(range(0, 4 * qb))
            fillers = list(fillers)
            nf = len(fillers)
            fpos = {}
            for i in range(nf):   # evenly over positions [2, nkb-1]
                p_i = 2 + ((i + 1) * (nkb - 2) - 1) // (nf + 1)
                fpos.setdefault(min(p_i, nkb - 1), []).append(fillers[i])
            emit_scores(kbs[0])
            emit_scores(kbs[1])
            for j in range(2, nkb):
                emit_scores(kbs[j])
                emit_av(kbs[j - 2], j - 2 == 0, False)
                for f in fpos.get(j, ()):
                    f()
            emit_av(kbs[nkb - 2], False, False)
            emit_av(kbs[nkb - 1], False, True)

            # normalize. A psum rows: [O_A | l_A]; B psum rows: [l_B | O_B].
            # cross-base PSUM reads avoid any partition-move DMA.
            onrm = til([128, 512], bf16, "onrm", 3)
            rc = til([128, 512], f32, "rc", 2)
            nc.vector.reciprocal(rc[64:128, :], oA[64:128, :])
            nc.vector.tensor_mul(onrm[0:64, :], oA[0:64, :], rc[64:128, :])
            nc.vector.reciprocal(rc[0:64, :], oB[0:64, :])
            nc.vector.tensor_mul(onrm[64:128, :], oB[64:128, :], rc[0:64, :])
            qcols = slice(qb * 512, (qb + 1) * 512)
            if p < 3:
                nc.sync.dma_start(og_send[p][:, qcols].opt(), onrm[:])
            else:
                nc.sync.dma_start(og_send3[qb][:].opt(), onrm[:])

        def exchange(p):
            if timing:
                nc.sync.dma_start(og_recv[p][0:128, :].opt(),
                                  og_send[p][:].opt())
                nc.sync.dma_start(og_recv[p][128:256, :].opt(),
                                  og_send[p][:].opt())
            else:
                nc.gpsimd.collective_compute(
                    "AllGather", mybir.AluOpType.bypass, replica_groups=GROUPS,
                    ins=[og_send[p][:].opt()], outs=[og_recv[p][:].opt()],
                )
            for g2 in range(2):
                t = til([128, S], bf16, "of", 8)
                nc.sync.dma_start(
                    t[:], og_recv[p][g2 * 128:(g2 + 1) * 128, :].opt())
                ofull[4 * g2 + p] = t

        def exchange3(qb):
            if timing:
                nc.sync.dma_start(og_recv3[qb][0:128, :].opt(),
                                  og_send3[qb][:].opt())
                nc.sync.dma_start(og_recv3[qb][128:256, :].opt(),
                                  og_send3[qb][:].opt())
            else:
                nc.gpsimd.collective_compute(
                    "AllGather", mybir.AluOpType.bypass, replica_groups=GROUPS,
                    ins=[og_send3[qb][:].opt()], outs=[og_recv3[qb][:].opt()],
                )
            if ofull[3] is None:
                ofull[3] = til([128, S], bf16, "of", 8)
                ofull[7] = til([128, S], bf16, "of", 8)
            qcols = slice(qb * 512, (qb + 1) * 512)
            for g2 in range(2):
                nc.sync.dma_start(
                    ofull[4 * g2 + 3][:, qcols],
                    og_recv3[qb][g2 * 128:(g2 + 1) * 128, :].opt())

        def outproj_st16(st16, box, half):
            # split into two 4-matmul halves so pair-3 filler slots are fine
            # grained enough to keep slack against the ACT pipeline
            qcols = slice(st16 * 128, (st16 + 1) * 128)
            if half == 0:
                ps = psp.tile([128, 512], f32, tag="proj", bufs=2)
                box[st16] = ps
                for i, dt in enumerate([0, 4, 1, 5]):
                    nc.tensor.matmul(box[st16][:], ofull[dt][:, qcols],
                                     ws(wo_t, dt, 0, 512),
                                     start=(i == 0), stop=False)
                return
            ps = box[st16]
            for i, dt in enumerate([2, 6, 3, 7]):
                nc.tensor.matmul(ps[:], ofull[dt][:, qcols],
                                 ws(wo_t, dt, 0, 512),
                                 start=False, stop=(i == 3))
            o_sb = til([128, SL], f32, "osb", 4)
            nc.vector.tensor_copy(o_sb[:], ps[:])
            nc.sync.dma_start(out[qcols, :], o_sb[:])

        opbox = {}

        def outproj_chunk(qb):
            for st16 in range(4 * qb, 4 * qb + 4):
                outproj_st16(st16, opbox, 0)
                outproj_st16(st16, opbox, 1)

        # ---------------- main pair loop ----------------
        # pair p+1's Q/K projection is interleaved into pair p's qb loop,
        # BEFORE attention so its rope drain isn't queued behind the norm ops
        # on the DVE (which wait for attention to finish). The output
        # projection chunks are decoupled from pair 3's qb loop with enough
        # slack to hide the 3-hop exchange chain.
        def set_qk2(p1, st, box):
            big_t = proj_phase2(p1, st, box)
            qtr[p1][st] = big_t[:, 0:512]
            ktr[p1][st] = big_t[:, 512:1024]

        def proj_fillers(p1, st):
            box = []
            return [lambda: proj_phase1(p1, st, box),
                    lambda: set_qk2(p1, st, box)]

        def set_qk(p1, st):
            box = []
            proj_phase1(p1, st, box)
            set_qk2(p1, st, box)

        set_qk(0, 0)
        for p in range(4):
            for qb in range(4):
                fillers = []
                if p == 0:
                    emit_v_quarter(qb)
                    if qb < 3:
                        fillers = proj_fillers(0, qb + 1)
                if p < 3:
                    # next-pair proj chains ride one qb later so the short
                    # qb=0 window never carries chains, and each chain has a
                    # full attention block of drain slack.
                    if qb == 3:
                        fillers += proj_fillers(p + 1, 2) + proj_fillers(p + 1, 3)
                    elif qb > 0:
                        fillers += proj_fillers(p + 1, qb - 1)
                elif qb >= 2:
                    fillers = [(lambda s, h: lambda: outproj_st16(s, opbox, h))(
                        st16, h)
                        for st16 in range(4 * (qb - 2), 4 * (qb - 2) + 4)
                        for h in (0, 1)]
                attention(p, qb, fillers)
                if p == 3:
                    exchange3(qb)
            if p < 3:
                exchange(p)
        outproj_chunk(2)
        outproj_chunk(3)


def rope_perm_rows(heads):
    rows = []
    for h in heads:
        rows += [h * DH + j for j in range(0, DH, 2)]
        rows += [h * DH + j for j in range(1, DH, 2)]
    return np.array(rows)


def prep_inputs(x, WQ, WK, WV, WO, token_positions):
    x = np.asarray(x, np.float32)
    WQ = np.asarray(WQ, np.float32)
    WK = np.asarray(WK, np.float32)
    WV = np.asarray(WV, np.float32)
    WO = np.asarray(WO, np.float32)
    pos = np.asarray(token_positions).astype(np.float32)
    bf = ml_dtypes.bfloat16

    permm = np.zeros((128, 128), np.float32)
    jj = np.arange(128)
    permm[jj, jj ^ 32] = 1.0

    r = np.arange(128)
    invf = (10000.0 ** (-(r % 32) / 32.0)).astype(np.float32)
    sign = np.where((r % 64) < 32, -1.0, 1.0).astype(np.float32)
    ang = pos[None, :] * invf[:, None]
    cosr = np.cos(ang).astype(np.float32)
    sinr = np.sin(ang * sign[:, None]).astype(np.float32)

    in_maps = []
    for c in range(NCORE):
        b, g = divmod(c, 2)
        heads = list(range(8 * g, 8 * g + 8))
        perm = rope_perm_rows(heads)
        rows = slice(8 * g * DH, (8 * g + 8) * DH)
        in_maps.append({
            "xT": np.ascontiguousarray(x[b].T).astype(bf),
            "wqT": np.ascontiguousarray(WQ[perm, :].T).astype(bf),
            "wkT": np.ascontiguousarray(WK[perm, :].T).astype(bf),
            "wvT": np.ascontiguousarray(WV.T[:, rows]).astype(bf),
            "woT": np.ascontiguousarray(WO.T[:, g * SL:(g + 1) * SL]).astype(bf),
            "permr": permm.astype(bf),
            "cosr": cosr,
            "sinr": sinr,
        })
    return in_maps


def assemble(results):
    B = NCORE // 2
    out = np.empty((B, S, D), np.float32)
    for b in range(B):
        out[b, :, 0:SL] = results[2 * b]["out"]
        out[b, :, SL:D] = results[2 * b + 1]["out"]
    return out


_NC = None


def _get_nc():
    global _NC
    if _NC is None:
        _NC = build()
    return _NC


def kernel(x, WQ, WK, WV, WO, token_positions):
    nc = _get_nc()
    in_maps = prep_inputs(x, WQ, WK, WV, WO, token_positions)
    res = run_bass_kernel_spmd(nc, in_maps, list(range(NCORE)))
    return assemble(res.results)
